# revision 6
# baseline (speedup 1.0000x reference)
"""Performer linear attention (nn_PerformerLinearAttention) — Trainium2 Bass kernel.

Sharding: 8 cores = (batch 2) x (sequence 4); core c handles batch c//4,
positions [(c%4)*1024, (c%4+1)*1024). Chunked causal linear attention with
chunk size 128; cross-core KV/z prefix state via an AllGather (groups
[[0..3],[4..7]]) of per-core KV totals, combined on-device with per-core
prefix-weight inputs (rank-agnostic program).

Phase order (program order ~ Tile priority):
  0 consts/hT/wv loads          1 v projection (pos-major)
  2 k path: proj+rope -> krot (resident), kpT (transient, for ksum),
    kp_pos, KV chains + snapshots, cc_in DMAs
  3 ksum scan, z, AllGather (fires early; later phases overlap it)
  4 q path: proj+rope, qpT (partition-stacked pairs), qsum rows
  5 attention: kpT recompute from krot, AT (masked), numT partial evicts
  6 prefix assembly, denominators, pass 2 (global KV term + normalize)
  7 output projection
"""
import numpy as np

import concourse.bacc as bacc
import concourse.mybir as mybir
import concourse.tile as tile
from concourse import bass_utils

FP32 = mybir.dt.float32
ADD = mybir.AluOpType.add
MULT = mybir.AluOpType.mult
MAX = mybir.AluOpType.max
BYPASS = mybir.AluOpType.bypass

NH, NKV, HD, NF = 16, 8, 64, 64
EPS_K, EPS_D = 1e-4, 1e-6
B, S, HM = 2, 4096, 1024
NCORES, GROUP = 8, 4
NPOS = S // GROUP            # 1024 positions per core
CH = 128
NCH = NPOS // CH             # 8 chunks
NPAIR = NH // 2              # 8 GQA pairs == kv heads


def build_nc():
    nc = bacc.Bacc("TRN2", target_bir_lowering=False, debug=False, num_devices=NCORES)

    # per-core inputs
    hT_d = nc.dram_tensor("hT", [HM, NPOS], FP32, kind="ExternalInput")
    cosT_d = nc.dram_tensor("cosT", [128, NPOS], FP32, kind="ExternalInput")
    sinTs_d = nc.dram_tensor("sinTs", [128, NPOS], FP32, kind="ExternalInput")
    wpref_d = nc.dram_tensor("wpref", [128, GROUP], FP32, kind="ExternalInput")
    # shared inputs
    wqkT_d = nc.dram_tensor("wqkT", [HM, 1536], FP32, kind="ExternalInput")
    wvT_d = nc.dram_tensor("wvT", [HM, 512], FP32, kind="ExternalInput")
    woT_d = nc.dram_tensor("woT", [HM, HM], FP32, kind="ExternalInput")
    projS2_d = nc.dram_tensor("projS2", [128, NF], FP32, kind="ExternalInput")
    permP_d = nc.dram_tensor("permP", [128, 128], FP32, kind="ExternalInput")
    tri_d = nc.dram_tensor("tri", [128, 128], FP32, kind="ExternalInput")
    ones128_d = nc.dram_tensor("ones128", [128, 1], FP32, kind="ExternalInput")
    onehot8_d = nc.dram_tensor("onehot8", [128, 512], FP32, kind="ExternalInput")
    # output
    out_d = nc.dram_tensor("outp", [NPOS, HM], FP32, kind="ExternalOutput")

    with tile.TileContext(nc) as tc:
        with (
            tc.tile_pool(name="consts", bufs=1) as cp,
            tc.tile_pool(name="main", bufs=1) as mp,
            tc.tile_pool(name="dram", bufs=1, space="DRAM") as dp,
            tc.tile_pool(name="psA", bufs=2, space="PSUM") as psA,
            tc.tile_pool(name="psB", bufs=4, space="PSUM") as psB,
            tc.tile_pool(name="psKV", bufs=2, space="PSUM") as psKV,
        ):
            # ---- constants ----
            cosT = cp.tile([128, NPOS], FP32, tag="cosT")
            nc.sync.dma_start(cosT[:], cosT_d[:])
            sinTs = cp.tile([128, NPOS], FP32, tag="sinTs")
            nc.sync.dma_start(sinTs[:], sinTs_d[:])
            projS2 = cp.tile([128, NF], FP32, tag="projS2")
            nc.sync.dma_start(projS2[:], projS2_d[:])
            permP = cp.tile([128, 128], FP32, tag="permP")
            nc.sync.dma_start(permP[:], permP_d[:])
            tri = cp.tile([128, 128], FP32, tag="tri")
            nc.sync.dma_start(tri[:], tri_d[:])
            ones128 = cp.tile([128, 1], FP32, tag="ones128")
            nc.sync.dma_start(ones128[:], ones128_d[:])
            onehot8 = cp.tile([128, 512], FP32, tag="onehot8")
            nc.sync.dma_start(onehot8[:], onehot8_d[:])
            wpref = cp.tile([128, GROUP], FP32, tag="wpref")
            nc.sync.dma_start(wpref[:], wpref_d[:])

            # ---- persistent (whole-kernel) arrays ----
            qpT = [mp.tile([128, NPOS], FP32, tag=f"qpT{p}", name=f"qpT{p}")
                   for p in range(NPAIR)]
            v_sb = [mp.tile([128, 512], FP32, tag=f"v{c}", name=f"v{c}")
                    for c in range(NCH)]
            krot = [mp.tile([128, NPOS], FP32, tag=f"krot{i}", name=f"krot{i}")
                    for i in range(4)]
            snaps = [[mp.tile([128, 64], FP32, tag=f"snap{g}_{c}",
                              name=f"snap{g}_{c}") for c in range(NCH)]
                     for g in range(NKV)]
            kz8 = mp.tile([8, NPOS], FP32, tag="kz8")    # ksum -> ztot (in-place)
            qre = mp.tile([8, NPOS], FP32, tag="qre")    # qsum_e -> r_e (in-place)
            qro = mp.tile([8, NPOS], FP32, tag="qro")    # qsum_o -> r_o (in-place)
            KVg = [mp.tile([128, 64], FP32, tag=f"kvg{g}", name=f"kvg{g}")
                   for g in range(NKV)]
            zgt = mp.tile([8, GROUP], FP32, tag="zgt")
            zpref = mp.tile([8, 1], FP32, tag="zpref")

            cc_in = dp.tile([520, 64], FP32, tag="cc_in")
            cc_out = dp.tile([GROUP * 520, 64], FP32, tag="cc_out")

            with (
                tc.tile_pool(name="phA", bufs=1) as pA,
                tc.tile_pool(name="trA", bufs=2) as tA,
                tc.tile_pool(name="wqks", bufs=4) as wqp,
            ):
                hTs = []
                for m in range(8):
                    t = pA.tile([128, NPOS], FP32, tag=f"hT{m}", name=f"hT{m}")
                    nc.sync.dma_start(t[:], hT_d[m * 128:(m + 1) * 128, :])
                    hTs.append(t)

                def proj_pair(rb, dest):
                    """QKV projection + RoPE for row-block rb (one head pair).
                    Writes the rope'd pair into `dest` [128, 1024]."""
                    for half in range(2):
                        ps = psA.tile([128, 512], FP32, tag="acc")
                        for m in range(8):
                            wq = wqp.tile([128, 128], FP32, tag="wqk")
                            nc.sync.dma_start(
                                wq[:], wqkT_d[m * 128:(m + 1) * 128,
                                              rb * 128:(rb + 1) * 128])
                            nc.tensor.matmul(ps[:], wq[:],
                                             hTs[m][:, half * 512:(half + 1) * 512],
                                             start=(m == 0), stop=(m == 7))
                        nc.scalar.copy(dest[:, half * 512:(half + 1) * 512], ps[:])
                    for half in range(2):
                        hs = slice(half * 512, (half + 1) * 512)
                        rps = psB.tile([128, 512], FP32, tag="ps")
                        nc.tensor.matmul(rps[:], permP[:], dest[:, hs],
                                         start=True, stop=True)
                        tmp = tA.tile([128, 512], FP32, tag="ropetmp")
                        nc.vector.tensor_tensor(out=tmp[:], in0=dest[:, hs],
                                                in1=cosT[:, hs], op=MULT)
                        nc.vector.tensor_tensor(out=rps[:], in0=rps[:],
                                                in1=sinTs[:, hs], op=MULT)
                        nc.vector.tensor_tensor(out=dest[:, hs], in0=tmp[:],
                                                in1=rps[:], op=ADD)

                def row_sum(dst_row_ap, rhs_ap, base):
                    """dst_row_ap [1, 512] (DRAM-side of a SBUF row via DMA) =
                    column sums of rhs_ap [64, 512] at partition base `base`."""
                    sps = psB.tile([1, 512], FP32, tag="ps")
                    nc.tensor.matmul(sps[:], ones128[base:base + 64, :], rhs_ap,
                                     start=True, stop=True)
                    stage = tA.tile([1, 512], FP32, tag="rowstage")
                    nc.scalar.copy(stage[:], sps[:])
                    nc.sync.dma_start(dst_row_ap, stage[:])

                # ---------- phase 1: v projection ----------
                with tc.tile_pool(name="phWV", bufs=1) as pwv, \
                     nc.named_scope("vproj"):
                    wv = []
                    for m in range(8):
                        t = pwv.tile([128, 512], FP32, tag=f"wv{m}", name=f"wv{m}")
                        nc.sync.dma_start(t[:], wvT_d[m * 128:(m + 1) * 128, :])
                        wv.append(t)
                    for c in range(NCH):
                        cs = slice(c * 128, (c + 1) * 128)
                        ps = psA.tile([128, 512], FP32, tag="acc")
                        for m in range(8):
                            nc.tensor.matmul(ps[:], hTs[m][:, cs], wv[m][:],
                                             start=(m == 0), stop=(m == 7))
                        nc.scalar.copy(v_sb[c][:], ps[:])

                # ---------- phase 2: k path ----------
                sc_k = nc.named_scope("kpath"); sc_k.__enter__()
                for i in range(4):          # rb = 8 + i ; kv heads 2i, 2i+1
                    proj_pair(8 + i, krot[i])
                    for hh in range(2):
                        g = 2 * i + hh
                        base = hh * 64
                        hsl = slice(base, base + 64)
                        # kpT (f-major) for ksum only — transient
                        kpt = tA.tile([64, NPOS], FP32, tag="kpt_tmp")
                        for half in range(2):
                            hs = slice(half * 512, (half + 1) * 512)
                            fps = psB.tile([64, 512], FP32, tag="ps")
                            nc.tensor.matmul(fps[:], projS2[hsl, :],
                                             krot[i][hsl, hs],
                                             start=True, stop=True)
                            nc.vector.tensor_scalar_max(kpt[:, hs], fps[:], 0.0)
                            row_sum(kz8[g:g + 1, hs], kpt[:, hs], 0)
                        # kp_pos + KV chain
                        kv_ps = psKV.tile([64, 64], FP32, tag="kv")
                        for c in range(NCH):
                            cs = slice(c * 128, (c + 1) * 128)
                            pps = psB.tile([128, 64], FP32, tag="ps")
                            nc.tensor.matmul(pps[:], krot[i][hsl, cs],
                                             projS2[hsl, :], start=True, stop=True)
                            kp_sb = tA.tile([128, 64], FP32, tag="kpos")
                            nc.vector.tensor_scalar_max(kp_sb[:], pps[:], 0.0)
                            nc.tensor.matmul(kv_ps[:], kp_sb[:],
                                             v_sb[c][:, g * 64:(g + 1) * 64],
                                             start=(c == 0), stop=(c == NCH - 1))
                            nc.scalar.copy(snaps[g][c][0:64, :], kv_ps[:])
                            nc.scalar.copy(snaps[g][c][64:128, :], kv_ps[:])
                        nc.sync.dma_start(cc_in[g * 64:(g + 1) * 64, :],
                                          snaps[g][NCH - 1][0:64, :])

                sc_k.__exit__(None, None, None)
                # ---------- phase 3: scan + collective ----------
                sc_s = nc.named_scope("scan_cc"); sc_s.__enter__()
                nc.vector.tensor_tensor_scan(
                    kz8[:, 0:512], kz8[:, 0:512], kz8[:, 0:512],
                    0.0, op0=ADD, op1=BYPASS)
                nc.vector.tensor_tensor_scan(
                    kz8[:, 512:1024], kz8[:, 512:1024], kz8[:, 512:1024],
                    kz8[:, 511:512], op0=ADD, op1=BYPASS)
                ztile = tA.tile([8, 1], FP32, tag="ztile")
                nc.vector.tensor_copy(ztile[:], kz8[:, NPOS - 1:NPOS])
                nc.sync.dma_start(cc_in[512:520, 0:1], ztile[:])
                nc.gpsimd.collective_compute(
                    "AllGather", BYPASS,
                    ins=[cc_in[:].opt()], outs=[cc_out[:].opt()],
                    replica_groups=[[0, 1, 2, 3], [4, 5, 6, 7]])

                sc_s.__exit__(None, None, None)
                # ---------- phase 4: q path ----------
                sc_q = nc.named_scope("qpath"); sc_q.__enter__()
                for g in range(NPAIR):
                    proj_pair(g, qpT[g])     # qpT[g] briefly holds rope'd q pair
                    for hh in range(2):
                        base = hh * 64
                        hsl = slice(base, base + 64)
                        for half in range(2):
                            hs = slice(half * 512, (half + 1) * 512)
                            fps = psB.tile([64, 512], FP32, tag="ps")
                            nc.tensor.matmul(fps[:], projS2[hsl, :], qpT[g][hsl, hs],
                                             start=True, stop=True)
                            nc.vector.tensor_scalar(qpT[g][hsl, hs], fps[:], 0.0,
                                                    EPS_K, op0=MAX, op1=ADD)
                    for hh in range(2):
                        base = hh * 64
                        qdst = qre if hh == 0 else qro
                        for half in range(2):
                            hs = slice(half * 512, (half + 1) * 512)
                            row_sum(qdst[g:g + 1, hs],
                                    qpT[g][base:base + 64, hs], base)

            sc_q.__exit__(None, None, None)
            # ---------- phases 5-7 ----------
            with (
                tc.tile_pool(name="phB", bufs=1) as pB,
                tc.tile_pool(name="trB", bufs=3) as tB,
            ):
                attnT = [pB.tile([128, NPOS], FP32, tag=f"attnT{p}",
                                 name=f"attnT{p}") for p in range(NPAIR)]

                # phase 5: attention (kpT recompute, AT, numerator partial)
                sc_a = nc.named_scope("attn"); sc_a.__enter__()
                for g in range(NKV):
                    i, hh = g // 2, g % 2
                    base = hh * 64
                    hsl = slice(base, base + 64)
                    kpt = tB.tile([128, NPOS], FP32, tag="kpt2", bufs=2)
                    for half in range(2):
                        hs = slice(half * 512, (half + 1) * 512)
                        fps = psB.tile([64, 512], FP32, tag="ps")
                        nc.tensor.matmul(fps[:], projS2[hsl, :], krot[i][hsl, hs],
                                         start=True, stop=True)
                        nc.vector.tensor_scalar_max(kpt[0:64, hs], fps[:], 0.0)
                        nc.vector.tensor_scalar_max(kpt[64:128, hs], fps[:], 0.0)
                    for c in range(NCH):
                        cs = slice(c * 128, (c + 1) * 128)
                        for hh2 in range(2):
                            b2 = hh2 * 64
                            h2sl = slice(b2, b2 + 64)
                            at = psB.tile([128, 128], FP32, tag="ps")
                            nc.tensor.matmul(at[:], kpt[h2sl, cs],
                                             qpT[g][h2sl, cs], start=True, stop=True)
                            ATm = tB.tile([128, 128], FP32, tag="atm")
                            nc.vector.tensor_tensor(out=ATm[:], in0=at[:],
                                                    in1=tri[:], op=MULT)
                            nps = psB.tile([64, 128], FP32, tag="ps")
                            nc.tensor.matmul(nps[:], v_sb[c][:, g * 64:(g + 1) * 64],
                                             ATm[:], start=True, stop=(c == 0))
                            if c > 0:
                                nc.tensor.matmul(nps[:], snaps[g][c - 1][h2sl, :],
                                                 qpT[g][h2sl, cs],
                                                 start=False, stop=True)
                            nc.scalar.copy(attnT[g][b2:b2 + 64, cs], nps[:])

                sc_a.__exit__(None, None, None)
                # phase 6a: prefix assembly
                sc_p = nc.named_scope("prefden"); sc_p.__enter__()
                for g in range(NKV):
                    for rho in range(GROUP):
                        gt = tB.tile([128, 64], FP32, tag="gath")
                        src = cc_out[rho * 520 + g * 64:rho * 520 + (g + 1) * 64, :]
                        nc.sync.dma_start(gt[0:64, :], src)
                        nc.sync.dma_start(gt[64:128, :], src)
                        if rho == 0:
                            nc.vector.tensor_scalar_mul(KVg[g][:], gt[:],
                                                        wpref[:, 0:1])
                        else:
                            nc.vector.scalar_tensor_tensor(
                                out=KVg[g][:], in0=gt[:],
                                scalar=wpref[:, rho:rho + 1],
                                in1=KVg[g][:], op0=MULT, op1=ADD)
                for rho in range(GROUP):
                    nc.sync.dma_start(zgt[:, rho:rho + 1],
                                      cc_out[rho * 520 + 512:rho * 520 + 520, 0:1])
                nc.vector.tensor_scalar_mul(zpref[:], zgt[:, 0:1], wpref[0:8, 0:1])
                for rho in range(1, GROUP):
                    nc.vector.scalar_tensor_tensor(
                        out=zpref[:], in0=zgt[:, rho:rho + 1],
                        scalar=wpref[0:8, rho:rho + 1], in1=zpref[:],
                        op0=MULT, op1=ADD)
                # phase 6b: denominators (in place)
                nc.vector.tensor_scalar_add(kz8[:], kz8[:], zpref[:, 0:1])
                for qt in (qre, qro):
                    nc.vector.tensor_tensor(out=qt[:], in0=qt[:], in1=kz8[:],
                                            op=MULT)
                    nc.vector.tensor_scalar_add(qt[:], qt[:], EPS_D)
                    nc.vector.reciprocal(qt[:], qt[:])

                sc_p.__exit__(None, None, None)
                # phase 6c: pass 2 — global KV term + normalize
                sc_2 = nc.named_scope("pass2"); sc_2.__enter__()
                for g in range(NKV):
                    for hh in range(2):
                        base = hh * 64
                        hsl = slice(base, base + 64)
                        rtile = qre if hh == 0 else qro
                        for half in range(2):
                            hs = slice(half * 512, (half + 1) * 512)
                            kvp = psB.tile([64, 512], FP32, tag="ps")
                            nc.tensor.matmul(kvp[:], KVg[g][hsl, :], qpT[g][hsl, hs],
                                             start=True, stop=True)
                            bps = psB.tile([64, 512], FP32, tag="ps")
                            nc.tensor.matmul(bps[:],
                                             onehot8[0:8, g * 64:(g + 1) * 64],
                                             rtile[:, hs],
                                             start=True, stop=True)
                            dst = attnT[g][hsl, hs]
                            nc.vector.tensor_tensor(out=dst, in0=dst, in1=kvp[:],
                                                    op=ADD)
                            nc.vector.tensor_tensor(out=dst, in0=dst, in1=bps[:],
                                                    op=MULT)

                sc_2.__exit__(None, None, None)
                # phase 7: output projection
                sc_o = nc.named_scope("oproj"); sc_o.__enter__()
                wo = []
                for t in range(8):
                    w = pB.tile([128, NPOS], FP32, tag=f"wo{t}", name=f"wo{t}")
                    nc.sync.dma_start(w[:], woT_d[t * 128:(t + 1) * 128, :])
                    wo.append(w)
                for c in range(NCH):
                    cs = slice(c * 128, (c + 1) * 128)
                    for mh in range(2):
                        ops_ = psA.tile([128, 512], FP32, tag="acc")
                        for t in range(8):
                            nc.tensor.matmul(ops_[:], attnT[t][:, cs],
                                             wo[t][:, mh * 512:(mh + 1) * 512],
                                             start=(t == 0), stop=(t == 7))
                        ost = tB.tile([128, 512], FP32, tag="ost")
                        nc.scalar.copy(ost[:], ops_[:])
                        nc.sync.dma_start(out_d[cs, mh * 512:(mh + 1) * 512], ost[:])
                sc_o.__exit__(None, None, None)

    nc.finalize()
    return nc


def _host_prep(cos, sin, W_qkv, W_o, proj):
    ratio = (NF ** -0.5) * (HD ** -0.5 + EPS_K)
    projS = np.ascontiguousarray((proj * ratio).T, dtype=np.float32)   # [d, f]
    projS2 = np.concatenate([projS, projS], axis=0)                    # [128, f]
    wqkT = np.ascontiguousarray(W_qkv[:1536].T, dtype=np.float32)      # [1024, 1536]
    wvT = np.ascontiguousarray(W_qkv[1536:].T, dtype=np.float32)       # [1024, 512]
    woT = np.ascontiguousarray(W_o.T, dtype=np.float32)                # [1024, 1024]
    sgn = np.concatenate([-np.ones(32, np.float32), np.ones(32, np.float32)])
    cosT1 = cos.T.astype(np.float32)                                   # [64, S]
    sinT1 = (sin.T * sgn[:, None]).astype(np.float32)
    cosT = np.concatenate([cosT1, cosT1], axis=0)                      # [128, S]
    sinTs = np.concatenate([sinT1, sinT1], axis=0)
    P = np.zeros((HD, HD), np.float32)
    for d in range(HD):
        P[(d + 32) % HD, d] = 1.0
    permP = np.zeros((128, 128), np.float32)
    permP[:64, :64] = P
    permP[64:, 64:] = P
    tri = np.triu(np.ones((CH, CH), np.float32))                       # keep j<=i
    ones128 = np.ones((128, 1), np.float32)
    onehot8 = np.zeros((128, 512), np.float32)
    for b0 in (0, 32, 64, 96):
        for g in range(8):
            onehot8[b0 + g, g * 64:(g + 1) * 64] = 1.0
    return dict(projS2=projS2, wqkT=wqkT, wvT=wvT, woT=woT, cosT=cosT,
                sinTs=sinTs, permP=permP, tri=tri, ones128=ones128,
                onehot8=onehot8)


_NC_CACHE = []


def kernel(**inputs):
    hidden = np.ascontiguousarray(np.asarray(inputs["hidden_states"], dtype=np.float32))
    cos = np.asarray(inputs["cos"], dtype=np.float32)
    sin = np.asarray(inputs["sin"], dtype=np.float32)
    W_qkv = np.asarray(inputs["W_qkv"], dtype=np.float32)
    W_o = np.asarray(inputs["W_o"], dtype=np.float32)
    proj = np.asarray(inputs["proj"], dtype=np.float32)

    prep = _host_prep(cos, sin, W_qkv, W_o, proj)
    shared = {k: prep[k] for k in ("wqkT", "wvT", "woT", "projS2", "permP",
                                   "tri", "ones128", "onehot8")}

    if not _NC_CACHE:
        _NC_CACHE.append(build_nc())
    nc = _NC_CACHE[0]

    in_maps = []
    for c in range(NCORES):
        b, rho = c // GROUP, c % GROUP
        sl = slice(rho * NPOS, (rho + 1) * NPOS)
        hT = np.ascontiguousarray(hidden[b, sl].T)
        wpref = np.zeros((128, GROUP), np.float32)
        wpref[:, :rho] = 1.0
        in_maps.append({"hT": hT,
                        "cosT": np.ascontiguousarray(prep["cosT"][:, sl]),
                        "sinTs": np.ascontiguousarray(prep["sinTs"][:, sl]),
                        "wpref": wpref, **shared})

    res = bass_utils.run_bass_kernel_spmd(nc, in_maps, core_ids=list(range(NCORES)))

    out = np.empty((B, S, HM), np.float32)
    for c in range(NCORES):
        b, rho = c // GROUP, c % GROUP
        out[b, rho * NPOS:(rho + 1) * NPOS, :] = res.results[c]["outp"]
    return out


# revision 7
# speedup vs baseline: 2.1401x; 2.1401x over previous
"""Performer linear attention (nn_PerformerLinearAttention) — Trainium2 Bass kernel.

Sharding: 8 cores = (batch 2) x (sequence 4); core c handles batch c//4,
positions [(c%4)*1024, (c%4+1)*1024). Chunked causal linear attention with
chunk size 128; cross-core KV/z prefix state via an AllGather (groups
[[0..3],[4..7]]) of per-core KV totals, combined on-device with per-core
prefix-weight inputs (rank-agnostic program).

Compute dtype: bf16 matmul streams with fp32 PSUM accumulation; the
denominator path (k/q feature sums, cumulative z scan, reciprocal) and the
collective payload stay fp32.

Phase order (program order ~ Tile priority):
  0 consts/hT/wv loads          1 v projection (pos-major)
  2 k path: proj+rope -> krot (resident), kpT (transient, for ksum),
    kp_pos, KV chains + snapshots, cc_in DMAs
  3 ksum scan, z, AllGather (fires early; later phases overlap it)
  4 per GQA pair g: q proj+rope+features+qsums, then attention
    (kpT recompute, masked AT, numerator partial)
  5 prefix assembly, denominators, pass 2 (global KV term + normalize)
  6 output projection
"""
import numpy as np
import ml_dtypes

import concourse.bacc as bacc
import concourse.mybir as mybir
import concourse.tile as tile
from concourse import bass_utils

FP32 = mybir.dt.float32
BF16 = mybir.dt.bfloat16
CDT = BF16                      # compute dtype for matmul streams
CNP = ml_dtypes.bfloat16        # numpy equivalent
ADD = mybir.AluOpType.add
MULT = mybir.AluOpType.mult
MAX = mybir.AluOpType.max
BYPASS = mybir.AluOpType.bypass

NH, NKV, HD, NF = 16, 8, 64, 64
EPS_K, EPS_D = 1e-4, 1e-6
B, S, HM = 2, 4096, 1024
NCORES, GROUP = 8, 4
NPOS = S // GROUP            # 1024 positions per core
CH = 128
NCH = NPOS // CH             # 8 chunks
NPAIR = NH // 2              # 8 GQA pairs == kv heads


def build_nc():
    nc = bacc.Bacc("TRN2", target_bir_lowering=False, debug=False, num_devices=NCORES)

    # per-core inputs
    hT_d = nc.dram_tensor("hT", [HM, NPOS], CDT, kind="ExternalInput")
    cosT_d = nc.dram_tensor("cosT", [128, NPOS], CDT, kind="ExternalInput")
    sinTs_d = nc.dram_tensor("sinTs", [128, NPOS], CDT, kind="ExternalInput")
    wpref_d = nc.dram_tensor("wpref", [128, GROUP], FP32, kind="ExternalInput")
    # shared inputs
    wqkT_d = nc.dram_tensor("wqkT", [HM, 1536], CDT, kind="ExternalInput")
    wvT_d = nc.dram_tensor("wvT", [HM, 512], CDT, kind="ExternalInput")
    woT_d = nc.dram_tensor("woT", [HM, HM], CDT, kind="ExternalInput")
    projS2_d = nc.dram_tensor("projS2", [128, NF], CDT, kind="ExternalInput")
    permP_d = nc.dram_tensor("permP", [128, 128], CDT, kind="ExternalInput")
    tri_d = nc.dram_tensor("tri", [128, 128], CDT, kind="ExternalInput")
    ones128_d = nc.dram_tensor("ones128", [128, 1], CDT, kind="ExternalInput")
    onehot8_d = nc.dram_tensor("onehot8", [8, 512], FP32, kind="ExternalInput")
    # output
    out_d = nc.dram_tensor("outp", [NPOS, HM], FP32, kind="ExternalOutput")

    with tile.TileContext(nc) as tc:
        with (
            tc.tile_pool(name="consts", bufs=1) as cp,
            tc.tile_pool(name="main", bufs=1) as mp,
            tc.tile_pool(name="trans", bufs=3) as tp,
            tc.tile_pool(name="wqks", bufs=4) as wqp,
            tc.tile_pool(name="dram", bufs=1, space="DRAM") as dp,
            tc.tile_pool(name="psA", bufs=2, space="PSUM") as psA,
            tc.tile_pool(name="psB", bufs=4, space="PSUM") as psB,
            tc.tile_pool(name="psKV", bufs=2, space="PSUM") as psKV,
        ):
            # ---- constants ----
            def const(name, shape, dt, src):
                t = cp.tile(shape, dt, tag=name, name=name)
                nc.sync.dma_start(t[:], src[:])
                return t

            cosT = const("cosT", [128, NPOS], CDT, cosT_d)
            sinTs = const("sinTs", [128, NPOS], CDT, sinTs_d)
            projS2 = const("projS2", [128, NF], CDT, projS2_d)
            permP = const("permP", [128, 128], CDT, permP_d)
            tri = const("tri", [128, 128], CDT, tri_d)
            ones128 = const("ones128", [128, 1], CDT, ones128_d)
            onehot8 = const("onehot8", [8, 512], FP32, onehot8_d)
            wpref = const("wpref", [128, GROUP], FP32, wpref_d)

            # ---- persistent arrays ----
            hTs = []
            for m in range(8):
                t = mp.tile([128, NPOS], CDT, tag=f"hT{m}", name=f"hT{m}")
                nc.sync.dma_start(t[:], hT_d[m * 128:(m + 1) * 128, :])
                hTs.append(t)
            qpT = [mp.tile([128, NPOS], CDT, tag=f"qpT{p}", name=f"qpT{p}")
                   for p in range(NPAIR)]
            v_sb = [mp.tile([128, 512], CDT, tag=f"v{c}", name=f"v{c}")
                    for c in range(NCH)]
            krot = [mp.tile([128, NPOS], CDT, tag=f"krot{i}", name=f"krot{i}")
                    for i in range(4)]
            snaps = [[mp.tile([128, 64], CDT, tag=f"snap{g}_{c}",
                              name=f"snap{g}_{c}") for c in range(NCH)]
                     for g in range(NKV)]
            attnT = [mp.tile([128, NPOS], CDT, tag=f"attnT{p}", name=f"attnT{p}")
                     for p in range(NPAIR)]
            wo = []
            for t_ in range(8):
                w = mp.tile([128, NPOS], CDT, tag=f"wo{t_}", name=f"wo{t_}")
                nc.sync.dma_start(w[:], woT_d[t_ * 128:(t_ + 1) * 128, :])
                wo.append(w)
            kz8 = mp.tile([8, NPOS], FP32, tag="kz8")    # ksum -> ztot (in-place)
            qre = mp.tile([8, NPOS], FP32, tag="qre")    # qsum_e -> r_e (in-place)
            qro = mp.tile([8, NPOS], FP32, tag="qro")    # qsum_o -> r_o (in-place)
            KVg = [mp.tile([128, 64], CDT, tag=f"kvg{g}", name=f"kvg{g}")
                   for g in range(NKV)]
            zgt = mp.tile([8, GROUP], FP32, tag="zgt")
            zpref = mp.tile([8, 1], FP32, tag="zpref")

            cc_in = dp.tile([520, 64], FP32, tag="cc_in")
            cc_out = dp.tile([GROUP * 520, 64], FP32, tag="cc_out")

            # ---------- helpers ----------
            def proj_pair(rb, dest):
                """QKV projection + RoPE for row-block rb (one head pair).
                Writes the rope'd pair into `dest` [128, 1024] (CDT)."""
                for half in range(2):
                    ps = psA.tile([128, 512], FP32, tag="acc")
                    for m in range(8):
                        wq = wqp.tile([128, 128], CDT, tag="wqk")
                        nc.sync.dma_start(
                            wq[:], wqkT_d[m * 128:(m + 1) * 128,
                                          rb * 128:(rb + 1) * 128])
                        nc.tensor.matmul(ps[:], wq[:],
                                         hTs[m][:, half * 512:(half + 1) * 512],
                                         start=(m == 0), stop=(m == 7))
                    nc.scalar.copy(dest[:, half * 512:(half + 1) * 512], ps[:])
                for half in range(2):
                    hs = slice(half * 512, (half + 1) * 512)
                    rps = psB.tile([128, 512], FP32, tag="ps")
                    nc.tensor.matmul(rps[:], permP[:], dest[:, hs],
                                     start=True, stop=True)
                    tmp = tp.tile([128, 512], CDT, tag="ropetmp")
                    nc.vector.tensor_tensor(out=tmp[:], in0=dest[:, hs],
                                            in1=cosT[:, hs], op=MULT)
                    nc.vector.tensor_tensor(out=rps[:], in0=rps[:],
                                            in1=sinTs[:, hs], op=MULT)
                    nc.vector.tensor_tensor(out=dest[:, hs], in0=tmp[:],
                                            in1=rps[:], op=ADD)

            def row_sum(dst_row_ap, rhs_ap, base):
                """dst row [1, 512] (via DMA) = column sums of rhs [64, 512]."""
                sps = psB.tile([1, 512], FP32, tag="ps")
                nc.tensor.matmul(sps[:], ones128[base:base + 64, :], rhs_ap,
                                 start=True, stop=True)
                stage = tp.tile([1, 512], FP32, tag="rowstage")
                nc.scalar.copy(stage[:], sps[:])
                nc.sync.dma_start(dst_row_ap, stage[:])

            # ---------- phase 1: v projection ----------
            with nc.named_scope("vproj"):
                wv = []
                for m in range(8):
                    t = mp.tile([128, 512], CDT, tag=f"wv{m}", name=f"wv{m}")
                    nc.sync.dma_start(t[:], wvT_d[m * 128:(m + 1) * 128, :])
                    wv.append(t)
                for c in range(NCH):
                    cs = slice(c * 128, (c + 1) * 128)
                    ps = psA.tile([128, 512], FP32, tag="acc")
                    for m in range(8):
                        nc.tensor.matmul(ps[:], hTs[m][:, cs], wv[m][:],
                                         start=(m == 0), stop=(m == 7))
                    nc.scalar.copy(v_sb[c][:], ps[:])

            # ---------- phase 2: k path ----------
            with nc.named_scope("kpath"):
                for i in range(4):          # rb = 8 + i ; kv heads 2i, 2i+1
                    proj_pair(8 + i, krot[i])
                    for hh in range(2):
                        g = 2 * i + hh
                        base = hh * 64
                        hsl = slice(base, base + 64)
                        kpt = tp.tile([64, NPOS], CDT, tag="kpt_tmp")
                        for half in range(2):
                            hs = slice(half * 512, (half + 1) * 512)
                            fps = psB.tile([64, 512], FP32, tag="ps")
                            nc.tensor.matmul(fps[:], projS2[hsl, :],
                                             krot[i][hsl, hs],
                                             start=True, stop=True)
                            nc.vector.tensor_scalar_max(kpt[:, hs], fps[:], 0.0)
                            row_sum(kz8[g:g + 1, hs], kpt[:, hs], 0)
                        kv_ps = psKV.tile([64, 64], FP32, tag="kv")
                        for c in range(NCH):
                            cs = slice(c * 128, (c + 1) * 128)
                            pps = psB.tile([128, 64], FP32, tag="ps")
                            nc.tensor.matmul(pps[:], krot[i][hsl, cs],
                                             projS2[hsl, :], start=True, stop=True)
                            kp_sb = tp.tile([128, 64], CDT, tag="kpos")
                            nc.vector.tensor_scalar_max(kp_sb[:], pps[:], 0.0)
                            nc.tensor.matmul(kv_ps[:], kp_sb[:],
                                             v_sb[c][:, g * 64:(g + 1) * 64],
                                             start=(c == 0), stop=(c == NCH - 1))
                            nc.scalar.copy(snaps[g][c][0:64, :], kv_ps[:])
                            nc.scalar.copy(snaps[g][c][64:128, :], kv_ps[:])
                        kvtot = tp.tile([64, 64], FP32, tag="kvtot")
                        nc.scalar.copy(kvtot[:], kv_ps[:])
                        nc.sync.dma_start(cc_in[g * 64:(g + 1) * 64, :], kvtot[:])

            # ---------- phase 3: scan + collective ----------
            with nc.named_scope("scan_cc"):
                nc.vector.tensor_tensor_scan(
                    kz8[:, 0:512], kz8[:, 0:512], kz8[:, 0:512],
                    0.0, op0=ADD, op1=BYPASS)
                nc.vector.tensor_tensor_scan(
                    kz8[:, 512:1024], kz8[:, 512:1024], kz8[:, 512:1024],
                    kz8[:, 511:512], op0=ADD, op1=BYPASS)
                ztile = tp.tile([8, 1], FP32, tag="ztile")
                nc.vector.tensor_copy(ztile[:], kz8[:, NPOS - 1:NPOS])
                nc.sync.dma_start(cc_in[512:520, 0:1], ztile[:])
                nc.gpsimd.collective_compute(
                    "AllGather", BYPASS,
                    ins=[cc_in[:].opt()], outs=[cc_out[:].opt()],
                    replica_groups=[[0, 1, 2, 3], [4, 5, 6, 7]])

            # ---------- phase 4: q path + attention, per pair ----------
            with nc.named_scope("qattn"):
                for g in range(NPAIR):
                    proj_pair(g, qpT[g])     # qpT[g] briefly holds rope'd q pair
                    for hh in range(2):
                        base = hh * 64
                        hsl = slice(base, base + 64)
                        for half in range(2):
                            hs = slice(half * 512, (half + 1) * 512)
                            fps = psB.tile([64, 512], FP32, tag="ps")
                            nc.tensor.matmul(fps[:], projS2[hsl, :], qpT[g][hsl, hs],
                                             start=True, stop=True)
                            nc.vector.tensor_scalar(qpT[g][hsl, hs], fps[:], 0.0,
                                                    EPS_K, op0=MAX, op1=ADD)
                    for hh in range(2):
                        base = hh * 64
                        qdst = qre if hh == 0 else qro
                        for half in range(2):
                            hs = slice(half * 512, (half + 1) * 512)
                            row_sum(qdst[g:g + 1, hs],
                                    qpT[g][base:base + 64, hs], base)
                    # attention for this pair
                    i, hh = g // 2, g % 2
                    hsl = slice(hh * 64, hh * 64 + 64)
                    kpt = tp.tile([128, NPOS], CDT, tag="kpt2", bufs=2)
                    for half in range(2):
                        hs = slice(half * 512, (half + 1) * 512)
                        fps = psB.tile([64, 512], FP32, tag="ps")
                        nc.tensor.matmul(fps[:], projS2[hsl, :], krot[i][hsl, hs],
                                         start=True, stop=True)
                        nc.vector.tensor_scalar_max(kpt[0:64, hs], fps[:], 0.0)
                        nc.vector.tensor_scalar_max(kpt[64:128, hs], fps[:], 0.0)
                    for c in range(NCH):
                        cs = slice(c * 128, (c + 1) * 128)
                        for hh2 in range(2):
                            b2 = hh2 * 64
                            h2sl = slice(b2, b2 + 64)
                            at = psB.tile([128, 128], FP32, tag="ps")
                            nc.tensor.matmul(at[:], kpt[h2sl, cs],
                                             qpT[g][h2sl, cs], start=True, stop=True)
                            ATm = tp.tile([128, 128], CDT, tag="atm")
                            nc.vector.tensor_tensor(out=ATm[:], in0=at[:],
                                                    in1=tri[:], op=MULT)
                            nps = psB.tile([64, 128], FP32, tag="ps")
                            nc.tensor.matmul(nps[:], v_sb[c][:, g * 64:(g + 1) * 64],
                                             ATm[:], start=True, stop=(c == 0))
                            if c > 0:
                                nc.tensor.matmul(nps[:], snaps[g][c - 1][h2sl, :],
                                                 qpT[g][h2sl, cs],
                                                 start=False, stop=True)
                            nc.scalar.copy(attnT[g][b2:b2 + 64, cs], nps[:])

            # ---------- phase 5: prefix assembly, denominators, pass 2 ----------
            with nc.named_scope("prefden"):
                for g in range(NKV):
                    kvacc = tp.tile([128, 64], FP32, tag="kvacc", bufs=2)
                    for rho in range(GROUP):
                        gt = tp.tile([128, 64], FP32, tag="gath")
                        src = cc_out[rho * 520 + g * 64:rho * 520 + (g + 1) * 64, :]
                        nc.sync.dma_start(gt[0:64, :], src)
                        nc.sync.dma_start(gt[64:128, :], src)
                        if rho == 0:
                            nc.vector.tensor_scalar_mul(kvacc[:], gt[:],
                                                        wpref[:, 0:1])
                        elif rho < GROUP - 1:
                            nc.vector.scalar_tensor_tensor(
                                out=kvacc[:], in0=gt[:],
                                scalar=wpref[:, rho:rho + 1],
                                in1=kvacc[:], op0=MULT, op1=ADD)
                        else:
                            nc.vector.scalar_tensor_tensor(
                                out=KVg[g][:], in0=gt[:],
                                scalar=wpref[:, rho:rho + 1],
                                in1=kvacc[:], op0=MULT, op1=ADD)
                for rho in range(GROUP):
                    nc.sync.dma_start(zgt[:, rho:rho + 1],
                                      cc_out[rho * 520 + 512:rho * 520 + 520, 0:1])
                nc.vector.tensor_scalar_mul(zpref[:], zgt[:, 0:1], wpref[0:8, 0:1])
                for rho in range(1, GROUP):
                    nc.vector.scalar_tensor_tensor(
                        out=zpref[:], in0=zgt[:, rho:rho + 1],
                        scalar=wpref[0:8, rho:rho + 1], in1=zpref[:],
                        op0=MULT, op1=ADD)
                nc.vector.tensor_scalar_add(kz8[:], kz8[:], zpref[:, 0:1])
                for qt in (qre, qro):
                    nc.vector.tensor_tensor(out=qt[:], in0=qt[:], in1=kz8[:],
                                            op=MULT)
                    nc.vector.tensor_scalar_add(qt[:], qt[:], EPS_D)
                    nc.vector.reciprocal(qt[:], qt[:])

            with nc.named_scope("pass2"):
                for g in range(NKV):
                    for hh in range(2):
                        hsl = slice(hh * 64, hh * 64 + 64)
                        rtile = qre if hh == 0 else qro
                        for half in range(2):
                            hs = slice(half * 512, (half + 1) * 512)
                            kvp = psB.tile([64, 512], FP32, tag="ps")
                            nc.tensor.matmul(kvp[:], KVg[g][hsl, :], qpT[g][hsl, hs],
                                             start=True, stop=True)
                            bps = psB.tile([64, 512], FP32, tag="ps")
                            nc.tensor.matmul(bps[:],
                                             onehot8[:, g * 64:(g + 1) * 64],
                                             rtile[:, hs],
                                             start=True, stop=True)
                            dst = attnT[g][hsl, hs]
                            nc.vector.tensor_tensor(out=dst, in0=dst, in1=kvp[:],
                                                    op=ADD)
                            nc.vector.tensor_tensor(out=dst, in0=dst, in1=bps[:],
                                                    op=MULT)

            # ---------- phase 6: output projection ----------
            with nc.named_scope("oproj"):
                for c in range(NCH):
                    cs = slice(c * 128, (c + 1) * 128)
                    for mh in range(2):
                        ops_ = psA.tile([128, 512], FP32, tag="acc")
                        for t_ in range(8):
                            nc.tensor.matmul(ops_[:], attnT[t_][:, cs],
                                             wo[t_][:, mh * 512:(mh + 1) * 512],
                                             start=(t_ == 0), stop=(t_ == 7))
                        ost = tp.tile([128, 512], FP32, tag="ost")
                        nc.scalar.copy(ost[:], ops_[:])
                        nc.sync.dma_start(out_d[cs, mh * 512:(mh + 1) * 512], ost[:])

    nc.finalize()
    return nc


def _host_prep(cos, sin, W_qkv, W_o, proj):
    ratio = (NF ** -0.5) * (HD ** -0.5 + EPS_K)
    projS = (proj * ratio).T.astype(CNP)                               # [d, f]
    projS2 = np.ascontiguousarray(np.concatenate([projS, projS], axis=0))
    wqkT = np.ascontiguousarray(W_qkv[:1536].T.astype(CNP))            # [1024, 1536]
    wvT = np.ascontiguousarray(W_qkv[1536:].T.astype(CNP))             # [1024, 512]
    woT = np.ascontiguousarray(W_o.T.astype(CNP))                      # [1024, 1024]
    sgn = np.concatenate([-np.ones(32, np.float32), np.ones(32, np.float32)])
    cosT1 = cos.T.astype(np.float32)
    sinT1 = (sin.T * sgn[:, None]).astype(np.float32)
    cosT = np.concatenate([cosT1, cosT1], axis=0).astype(CNP)          # [128, S]
    sinTs = np.concatenate([sinT1, sinT1], axis=0).astype(CNP)
    P = np.zeros((HD, HD), np.float32)
    for d in range(HD):
        P[(d + 32) % HD, d] = 1.0
    permP = np.zeros((128, 128), np.float32)
    permP[:64, :64] = P
    permP[64:, 64:] = P
    tri = np.triu(np.ones((CH, CH), np.float32)).astype(CNP)
    ones128 = np.ones((128, 1), CNP)
    onehot8 = np.zeros((8, 512), np.float32)
    for g in range(8):
        onehot8[g, g * 64:(g + 1) * 64] = 1.0
    return dict(projS2=projS2, wqkT=wqkT, wvT=wvT, woT=woT, cosT=cosT,
                sinTs=sinTs, permP=permP.astype(CNP), tri=tri, ones128=ones128,
                onehot8=onehot8)


_NC_CACHE = []


def kernel(**inputs):
    hidden = np.asarray(inputs["hidden_states"], dtype=np.float32)
    cos = np.asarray(inputs["cos"], dtype=np.float32)
    sin = np.asarray(inputs["sin"], dtype=np.float32)
    W_qkv = np.asarray(inputs["W_qkv"], dtype=np.float32)
    W_o = np.asarray(inputs["W_o"], dtype=np.float32)
    proj = np.asarray(inputs["proj"], dtype=np.float32)

    prep = _host_prep(cos, sin, W_qkv, W_o, proj)
    shared = {k: prep[k] for k in ("wqkT", "wvT", "woT", "projS2", "permP",
                                   "tri", "ones128", "onehot8")}

    if not _NC_CACHE:
        _NC_CACHE.append(build_nc())
    nc = _NC_CACHE[0]

    in_maps = []
    for c in range(NCORES):
        b, rho = c // GROUP, c % GROUP
        sl = slice(rho * NPOS, (rho + 1) * NPOS)
        hT = np.ascontiguousarray(hidden[b, sl].T.astype(CNP))
        wpref = np.zeros((128, GROUP), np.float32)
        wpref[:, :rho] = 1.0
        in_maps.append({"hT": hT,
                        "cosT": np.ascontiguousarray(prep["cosT"][:, sl]),
                        "sinTs": np.ascontiguousarray(prep["sinTs"][:, sl]),
                        "wpref": wpref, **shared})

    res = bass_utils.run_bass_kernel_spmd(nc, in_maps, core_ids=list(range(NCORES)))

    out = np.empty((B, S, HM), np.float32)
    for c in range(NCORES):
        b, rho = c // GROUP, c % GROUP
        out[b, rho * NPOS:(rho + 1) * NPOS, :] = res.results[c]["outp"]
    return out


# revision 8
# speedup vs baseline: 2.2736x; 1.0624x over previous
"""Performer linear attention (nn_PerformerLinearAttention) — Trainium2 Bass kernel.

Sharding: 8 cores = (batch 2) x (sequence 4); core c handles batch c//4,
positions [(c%4)*1024, (c%4+1)*1024). Chunked causal linear attention with
chunk size 128; cross-core KV/z prefix state via an AllGather (groups
[[0..3],[4..7]]) of per-core KV totals, combined on-device with per-core
prefix-weight inputs (rank-agnostic program).

Compute dtype: bf16 matmul streams with fp32 PSUM accumulation; the
denominator path (k/q feature sums, cumulative z scan, reciprocal) and the
collective payload stay fp32.

Phase order (program order ~ Tile priority):
  0 consts/hT/wv loads          1 v projection (pos-major)
  2 k path: proj+rope -> krot (resident), kpT (transient, for ksum),
    kp_pos, KV chains + snapshots, cc_in DMAs
  3 ksum scan, z, AllGather (fires early; later phases overlap it)
  4 per GQA pair g: q proj+rope+features+qsums, then attention
    (kpT recompute, masked AT, numerator partial)
  5 prefix assembly, denominators, pass 2 (global KV term + normalize)
  6 output projection
"""
import numpy as np
import ml_dtypes

import concourse.bacc as bacc
import concourse.mybir as mybir
import concourse.tile as tile
from concourse import bass_utils

FP32 = mybir.dt.float32
BF16 = mybir.dt.bfloat16
CDT = BF16                      # compute dtype for matmul streams
CNP = ml_dtypes.bfloat16        # numpy equivalent
ADD = mybir.AluOpType.add
MULT = mybir.AluOpType.mult
MAX = mybir.AluOpType.max
BYPASS = mybir.AluOpType.bypass

NH, NKV, HD, NF = 16, 8, 64, 64
EPS_K, EPS_D = 1e-4, 1e-6
B, S, HM = 2, 4096, 1024
NCORES, GROUP = 8, 4
NPOS = S // GROUP            # 1024 positions per core
CH = 128
NCH = NPOS // CH             # 8 chunks
NPAIR = NH // 2              # 8 GQA pairs == kv heads


def build_nc():
    nc = bacc.Bacc("TRN2", target_bir_lowering=False, debug=False, num_devices=NCORES)

    # per-core inputs
    hT_d = nc.dram_tensor("hT", [HM, NPOS], CDT, kind="ExternalInput")
    cosT_d = nc.dram_tensor("cosT", [128, NPOS], CDT, kind="ExternalInput")
    sinTs_d = nc.dram_tensor("sinTs", [128, NPOS], CDT, kind="ExternalInput")
    wpref_d = nc.dram_tensor("wpref", [128, GROUP], FP32, kind="ExternalInput")
    # shared inputs
    wqkT_d = nc.dram_tensor("wqkT", [HM, 1536], CDT, kind="ExternalInput")
    wvT_d = nc.dram_tensor("wvT", [HM, 512], CDT, kind="ExternalInput")
    woT_d = nc.dram_tensor("woT", [HM, HM], CDT, kind="ExternalInput")
    projS2_d = nc.dram_tensor("projS2", [128, NF], CDT, kind="ExternalInput")
    permP_d = nc.dram_tensor("permP", [128, 128], CDT, kind="ExternalInput")
    tri_d = nc.dram_tensor("tri", [128, 128], CDT, kind="ExternalInput")
    ones128_d = nc.dram_tensor("ones128", [128, 1], CDT, kind="ExternalInput")
    onehot8_d = nc.dram_tensor("onehot8", [8, 512], FP32, kind="ExternalInput")
    # output
    out_d = nc.dram_tensor("outp", [NPOS, HM], FP32, kind="ExternalOutput")

    with tile.TileContext(nc) as tc:
        with (
            tc.tile_pool(name="consts", bufs=1) as cp,
            tc.tile_pool(name="main", bufs=1) as mp,
            tc.tile_pool(name="trans", bufs=3) as tp,
            tc.tile_pool(name="dram", bufs=1, space="DRAM") as dp,
            tc.tile_pool(name="psA", bufs=2, space="PSUM") as psA,
            tc.tile_pool(name="psB", bufs=4, space="PSUM") as psB,
            tc.tile_pool(name="psKV", bufs=2, space="PSUM") as psKV,
        ):
            # ---- constants ----
            def const(name, shape, dt, src):
                t = cp.tile(shape, dt, tag=name, name=name)
                nc.sync.dma_start(t[:], src[:])
                return t

            cosT = const("cosT", [128, NPOS], CDT, cosT_d)
            sinTs = const("sinTs", [128, NPOS], CDT, sinTs_d)
            projS2 = const("projS2", [128, NF], CDT, projS2_d)
            permP = const("permP", [128, 128], CDT, permP_d)
            tri = const("tri", [128, 128], CDT, tri_d)
            ones128 = const("ones128", [128, 1], CDT, ones128_d)
            onehot8 = const("onehot8", [8, 512], FP32, onehot8_d)
            wpref = const("wpref", [128, GROUP], FP32, wpref_d)

            # ---- persistent arrays ----
            hTs = []
            for m in range(8):
                t = mp.tile([128, NPOS], CDT, tag=f"hT{m}", name=f"hT{m}")
                nc.sync.dma_start(t[:], hT_d[m * 128:(m + 1) * 128, :])
                hTs.append(t)
            wqk = []
            for m in range(8):
                t = mp.tile([128, 1536], CDT, tag=f"wqk{m}", name=f"wqk{m}")
                nc.sync.dma_start(t[:], wqkT_d[m * 128:(m + 1) * 128, :])
                wqk.append(t)
            qpT = [mp.tile([128, NPOS], CDT, tag=f"qpT{p}", name=f"qpT{p}")
                   for p in range(NPAIR)]
            v_sb = [mp.tile([128, 512], CDT, tag=f"v{c}", name=f"v{c}")
                    for c in range(NCH)]
            krot = [mp.tile([128, NPOS], CDT, tag=f"krot{i}", name=f"krot{i}")
                    for i in range(4)]
            snaps = [[mp.tile([128, 64], CDT, tag=f"snap{g}_{c}",
                              name=f"snap{g}_{c}") for c in range(NCH)]
                     for g in range(NKV)]
            attnT = [mp.tile([128, NPOS], CDT, tag=f"attnT{p}", name=f"attnT{p}")
                     for p in range(NPAIR)]
            wo = []
            for t_ in range(8):
                w = mp.tile([128, NPOS], CDT, tag=f"wo{t_}", name=f"wo{t_}")
                nc.sync.dma_start(w[:], woT_d[t_ * 128:(t_ + 1) * 128, :])
                wo.append(w)
            kz8 = mp.tile([8, NPOS], FP32, tag="kz8")    # ksum -> ztot (in-place)
            qre = mp.tile([8, NPOS], FP32, tag="qre")    # qsum_e -> r_e (in-place)
            qro = mp.tile([8, NPOS], FP32, tag="qro")    # qsum_o -> r_o (in-place)
            KVg = [mp.tile([128, 64], CDT, tag=f"kvg{g}", name=f"kvg{g}")
                   for g in range(NKV)]
            zgt = mp.tile([8, GROUP], FP32, tag="zgt")
            zpref = mp.tile([8, 1], FP32, tag="zpref")

            cc_in = dp.tile([520, 64], FP32, tag="cc_in")
            cc_out = dp.tile([GROUP * 520, 64], FP32, tag="cc_out")

            # ---------- helpers ----------
            def proj_pair(rb, dest):
                """QKV projection + RoPE for row-block rb (one head pair).
                Writes the rope'd pair into `dest` [128, 1024] (CDT)."""
                for half in range(2):
                    ps = psA.tile([128, 512], FP32, tag="acc")
                    for m in range(8):
                        nc.tensor.matmul(ps[:], wqk[m][:, rb * 128:(rb + 1) * 128],
                                         hTs[m][:, half * 512:(half + 1) * 512],
                                         start=(m == 0), stop=(m == 7))
                    nc.scalar.copy(dest[:, half * 512:(half + 1) * 512], ps[:])
                for half in range(2):
                    hs = slice(half * 512, (half + 1) * 512)
                    rps = psB.tile([128, 512], FP32, tag="ps")
                    nc.tensor.matmul(rps[:], permP[:], dest[:, hs],
                                     start=True, stop=True)
                    tmp = tp.tile([128, 512], CDT, tag="ropetmp")
                    nc.vector.tensor_tensor(out=tmp[:], in0=dest[:, hs],
                                            in1=cosT[:, hs], op=MULT)
                    nc.vector.tensor_tensor(out=rps[:], in0=rps[:],
                                            in1=sinTs[:, hs], op=MULT)
                    nc.vector.tensor_tensor(out=dest[:, hs], in0=tmp[:],
                                            in1=rps[:], op=ADD)

            def row_sum(dst_row_ap, rhs_aps, bases):
                """dst row [1, 1024] (via one DMA) = column sums of two
                [64, 512] halves."""
                stage = tp.tile([1, NPOS], FP32, tag="rowstage")
                for half, (rhs_ap, base) in enumerate(zip(rhs_aps, bases)):
                    sps = psB.tile([1, 512], FP32, tag="ps")
                    nc.tensor.matmul(sps[:], ones128[base:base + 64, :], rhs_ap,
                                     start=True, stop=True)
                    nc.scalar.copy(stage[:, half * 512:(half + 1) * 512], sps[:])
                nc.sync.dma_start(dst_row_ap, stage[:])

            # ---------- phase 1: v projection ----------
            with nc.named_scope("vproj"):
                wv = []
                for m in range(8):
                    t = mp.tile([128, 512], CDT, tag=f"wv{m}", name=f"wv{m}")
                    nc.sync.dma_start(t[:], wvT_d[m * 128:(m + 1) * 128, :])
                    wv.append(t)
                for c in range(NCH):
                    cs = slice(c * 128, (c + 1) * 128)
                    ps = psA.tile([128, 512], FP32, tag="acc")
                    for m in range(8):
                        nc.tensor.matmul(ps[:], hTs[m][:, cs], wv[m][:],
                                         start=(m == 0), stop=(m == 7))
                    nc.scalar.copy(v_sb[c][:], ps[:])

            # ---------- phase 2: k path ----------
            with nc.named_scope("kpath"):
                for i in range(4):          # rb = 8 + i ; kv heads 2i, 2i+1
                    proj_pair(8 + i, krot[i])
                    for hh in range(2):
                        g = 2 * i + hh
                        base = hh * 64
                        hsl = slice(base, base + 64)
                        kpt = tp.tile([64, NPOS], CDT, tag="kpt_tmp")
                        for half in range(2):
                            hs = slice(half * 512, (half + 1) * 512)
                            fps = psB.tile([64, 512], FP32, tag="ps")
                            nc.tensor.matmul(fps[:], projS2[hsl, :],
                                             krot[i][hsl, hs],
                                             start=True, stop=True)
                            nc.vector.tensor_scalar_max(kpt[:, hs], fps[:], 0.0)
                        row_sum(kz8[g:g + 1, :],
                                (kpt[:, 0:512], kpt[:, 512:1024]), (0, 0))
                        kv_ps = psKV.tile([64, 64], FP32, tag="kv")
                        for c in range(NCH):
                            cs = slice(c * 128, (c + 1) * 128)
                            pps = psB.tile([128, 64], FP32, tag="ps")
                            nc.tensor.matmul(pps[:], krot[i][hsl, cs],
                                             projS2[hsl, :], start=True, stop=True)
                            kp_sb = tp.tile([128, 64], CDT, tag="kpos")
                            nc.vector.tensor_scalar_max(kp_sb[:], pps[:], 0.0)
                            nc.tensor.matmul(kv_ps[:], kp_sb[:],
                                             v_sb[c][:, g * 64:(g + 1) * 64],
                                             start=(c == 0), stop=(c == NCH - 1))
                            nc.scalar.copy(snaps[g][c][0:64, :], kv_ps[:])
                            nc.scalar.copy(snaps[g][c][64:128, :], kv_ps[:])
                        kvtot = tp.tile([64, 64], FP32, tag="kvtot")
                        nc.scalar.copy(kvtot[:], kv_ps[:])
                        nc.sync.dma_start(cc_in[g * 64:(g + 1) * 64, :], kvtot[:])

            # ---------- phase 3: scan + collective ----------
            with nc.named_scope("scan_cc"):
                nc.vector.tensor_tensor_scan(
                    kz8[:, 0:512], kz8[:, 0:512], kz8[:, 0:512],
                    0.0, op0=ADD, op1=BYPASS)
                nc.vector.tensor_tensor_scan(
                    kz8[:, 512:1024], kz8[:, 512:1024], kz8[:, 512:1024],
                    kz8[:, 511:512], op0=ADD, op1=BYPASS)
                ztile = tp.tile([8, 1], FP32, tag="ztile")
                nc.vector.tensor_copy(ztile[:], kz8[:, NPOS - 1:NPOS])
                nc.sync.dma_start(cc_in[512:520, 0:1], ztile[:])
                nc.gpsimd.collective_compute(
                    "AllGather", BYPASS,
                    ins=[cc_in[:].opt()], outs=[cc_out[:].opt()],
                    replica_groups=[[0, 1, 2, 3], [4, 5, 6, 7]])

            # ---------- phase 4: q path + attention, per pair ----------
            with nc.named_scope("qattn"):
                for g in range(NPAIR):
                    proj_pair(g, qpT[g])     # qpT[g] briefly holds rope'd q pair
                    for hh in range(2):
                        base = hh * 64
                        hsl = slice(base, base + 64)
                        for half in range(2):
                            hs = slice(half * 512, (half + 1) * 512)
                            fps = psB.tile([64, 512], FP32, tag="ps")
                            nc.tensor.matmul(fps[:], projS2[hsl, :], qpT[g][hsl, hs],
                                             start=True, stop=True)
                            nc.vector.tensor_scalar(qpT[g][hsl, hs], fps[:], 0.0,
                                                    EPS_K, op0=MAX, op1=ADD)
                    for hh in range(2):
                        base = hh * 64
                        qdst = qre if hh == 0 else qro
                        row_sum(qdst[g:g + 1, :],
                                (qpT[g][base:base + 64, 0:512],
                                 qpT[g][base:base + 64, 512:1024]),
                                (base, base))
                    # attention for this pair
                    i, hh = g // 2, g % 2
                    hsl = slice(hh * 64, hh * 64 + 64)
                    kpt = tp.tile([128, NPOS], CDT, tag="kpt2", bufs=2)
                    for half in range(2):
                        hs = slice(half * 512, (half + 1) * 512)
                        fps = psB.tile([64, 512], FP32, tag="ps")
                        nc.tensor.matmul(fps[:], projS2[hsl, :], krot[i][hsl, hs],
                                         start=True, stop=True)
                        nc.vector.tensor_scalar_max(kpt[0:64, hs], fps[:], 0.0)
                        nc.vector.tensor_scalar_max(kpt[64:128, hs], fps[:], 0.0)
                    for c in range(NCH):
                        cs = slice(c * 128, (c + 1) * 128)
                        for hh2 in range(2):
                            b2 = hh2 * 64
                            h2sl = slice(b2, b2 + 64)
                            at = psB.tile([128, 128], FP32, tag="ps")
                            nc.tensor.matmul(at[:], kpt[h2sl, cs],
                                             qpT[g][h2sl, cs], start=True, stop=True)
                            ATm = tp.tile([128, 128], CDT, tag="atm")
                            nc.vector.tensor_tensor(out=ATm[:], in0=at[:],
                                                    in1=tri[:], op=MULT)
                            nps = psB.tile([64, 128], FP32, tag="ps")
                            nc.tensor.matmul(nps[:], v_sb[c][:, g * 64:(g + 1) * 64],
                                             ATm[:], start=True, stop=(c == 0))
                            if c > 0:
                                nc.tensor.matmul(nps[:], snaps[g][c - 1][h2sl, :],
                                                 qpT[g][h2sl, cs],
                                                 start=False, stop=True)
                            nc.scalar.copy(attnT[g][b2:b2 + 64, cs], nps[:])

            # ---------- phase 5: prefix assembly, denominators, pass 2 ----------
            with nc.named_scope("prefden"):
                for g in range(NKV):
                    kvacc = tp.tile([128, 64], FP32, tag="kvacc", bufs=2)
                    for rho in range(GROUP):
                        gt = tp.tile([128, 64], FP32, tag="gath")
                        src = cc_out[rho * 520 + g * 64:rho * 520 + (g + 1) * 64, :]
                        nc.gpsimd.dma_start(gt[0:64, :], src)
                        nc.gpsimd.dma_start(gt[64:128, :], src)
                        if rho == 0:
                            nc.vector.tensor_scalar_mul(kvacc[:], gt[:],
                                                        wpref[:, 0:1])
                        elif rho < GROUP - 1:
                            nc.vector.scalar_tensor_tensor(
                                out=kvacc[:], in0=gt[:],
                                scalar=wpref[:, rho:rho + 1],
                                in1=kvacc[:], op0=MULT, op1=ADD)
                        else:
                            nc.vector.scalar_tensor_tensor(
                                out=KVg[g][:], in0=gt[:],
                                scalar=wpref[:, rho:rho + 1],
                                in1=kvacc[:], op0=MULT, op1=ADD)
                for rho in range(GROUP):
                    nc.gpsimd.dma_start(zgt[:, rho:rho + 1],
                                      cc_out[rho * 520 + 512:rho * 520 + 520, 0:1])
                nc.vector.tensor_scalar_mul(zpref[:], zgt[:, 0:1], wpref[0:8, 0:1])
                for rho in range(1, GROUP):
                    nc.vector.scalar_tensor_tensor(
                        out=zpref[:], in0=zgt[:, rho:rho + 1],
                        scalar=wpref[0:8, rho:rho + 1], in1=zpref[:],
                        op0=MULT, op1=ADD)
                nc.vector.tensor_scalar_add(kz8[:], kz8[:], zpref[:, 0:1])
                for qt in (qre, qro):
                    nc.vector.tensor_tensor(out=qt[:], in0=qt[:], in1=kz8[:],
                                            op=MULT)
                    nc.vector.tensor_scalar_add(qt[:], qt[:], EPS_D)
                    nc.vector.reciprocal(qt[:], qt[:])

            with nc.named_scope("pass2"):
                for g in range(NKV):
                    for hh in range(2):
                        hsl = slice(hh * 64, hh * 64 + 64)
                        rtile = qre if hh == 0 else qro
                        rst = tp.tile([1, NPOS], FP32, tag="rst")
                        nc.sync.dma_start(rst[:], rtile[g:g + 1, :])
                        rbc = tp.tile([128, NPOS], FP32, tag="rbc", bufs=2)
                        nc.gpsimd.partition_broadcast(rbc[:], rst[:])
                        for half in range(2):
                            hs = slice(half * 512, (half + 1) * 512)
                            kvp = psB.tile([64, 512], FP32, tag="ps")
                            nc.tensor.matmul(kvp[:], KVg[g][hsl, :], qpT[g][hsl, hs],
                                             start=True, stop=True)
                            dst = attnT[g][hsl, hs]
                            nc.vector.tensor_tensor(out=dst, in0=dst, in1=kvp[:],
                                                    op=ADD)
                            nc.vector.tensor_tensor(out=dst, in0=dst,
                                                    in1=rbc[hsl, hs], op=MULT)

            # ---------- phase 6: output projection ----------
            with nc.named_scope("oproj"):
                for c in range(NCH):
                    cs = slice(c * 128, (c + 1) * 128)
                    for mh in range(2):
                        ops_ = psA.tile([128, 512], FP32, tag="acc")
                        for t_ in range(8):
                            nc.tensor.matmul(ops_[:], attnT[t_][:, cs],
                                             wo[t_][:, mh * 512:(mh + 1) * 512],
                                             start=(t_ == 0), stop=(t_ == 7))
                        ost = tp.tile([128, 512], FP32, tag="ost")
                        nc.scalar.copy(ost[:], ops_[:])
                        nc.sync.dma_start(out_d[cs, mh * 512:(mh + 1) * 512], ost[:])

    nc.finalize()
    return nc


def _host_prep(cos, sin, W_qkv, W_o, proj):
    ratio = (NF ** -0.5) * (HD ** -0.5 + EPS_K)
    projS = (proj * ratio).T.astype(CNP)                               # [d, f]
    projS2 = np.ascontiguousarray(np.concatenate([projS, projS], axis=0))
    wqkT = np.ascontiguousarray(W_qkv[:1536].T.astype(CNP))            # [1024, 1536]
    wvT = np.ascontiguousarray(W_qkv[1536:].T.astype(CNP))             # [1024, 512]
    woT = np.ascontiguousarray(W_o.T.astype(CNP))                      # [1024, 1024]
    sgn = np.concatenate([-np.ones(32, np.float32), np.ones(32, np.float32)])
    cosT1 = cos.T.astype(np.float32)
    sinT1 = (sin.T * sgn[:, None]).astype(np.float32)
    cosT = np.concatenate([cosT1, cosT1], axis=0).astype(CNP)          # [128, S]
    sinTs = np.concatenate([sinT1, sinT1], axis=0).astype(CNP)
    P = np.zeros((HD, HD), np.float32)
    for d in range(HD):
        P[(d + 32) % HD, d] = 1.0
    permP = np.zeros((128, 128), np.float32)
    permP[:64, :64] = P
    permP[64:, 64:] = P
    tri = np.triu(np.ones((CH, CH), np.float32)).astype(CNP)
    ones128 = np.ones((128, 1), CNP)
    onehot8 = np.zeros((8, 512), np.float32)
    for g in range(8):
        onehot8[g, g * 64:(g + 1) * 64] = 1.0
    return dict(projS2=projS2, wqkT=wqkT, wvT=wvT, woT=woT, cosT=cosT,
                sinTs=sinTs, permP=permP.astype(CNP), tri=tri, ones128=ones128,
                onehot8=onehot8)


_NC_CACHE = []


def kernel(**inputs):
    hidden = np.asarray(inputs["hidden_states"], dtype=np.float32)
    cos = np.asarray(inputs["cos"], dtype=np.float32)
    sin = np.asarray(inputs["sin"], dtype=np.float32)
    W_qkv = np.asarray(inputs["W_qkv"], dtype=np.float32)
    W_o = np.asarray(inputs["W_o"], dtype=np.float32)
    proj = np.asarray(inputs["proj"], dtype=np.float32)

    prep = _host_prep(cos, sin, W_qkv, W_o, proj)
    shared = {k: prep[k] for k in ("wqkT", "wvT", "woT", "projS2", "permP",
                                   "tri", "ones128", "onehot8")}

    if not _NC_CACHE:
        _NC_CACHE.append(build_nc())
    nc = _NC_CACHE[0]

    in_maps = []
    for c in range(NCORES):
        b, rho = c // GROUP, c % GROUP
        sl = slice(rho * NPOS, (rho + 1) * NPOS)
        hT = np.ascontiguousarray(hidden[b, sl].T.astype(CNP))
        wpref = np.zeros((128, GROUP), np.float32)
        wpref[:, :rho] = 1.0
        in_maps.append({"hT": hT,
                        "cosT": np.ascontiguousarray(prep["cosT"][:, sl]),
                        "sinTs": np.ascontiguousarray(prep["sinTs"][:, sl]),
                        "wpref": wpref, **shared})

    res = bass_utils.run_bass_kernel_spmd(nc, in_maps, core_ids=list(range(NCORES)))

    out = np.empty((B, S, HM), np.float32)
    for c in range(NCORES):
        b, rho = c // GROUP, c % GROUP
        out[b, rho * NPOS:(rho + 1) * NPOS, :] = res.results[c]["outp"]
    return out


# revision 9
# speedup vs baseline: 2.8196x; 1.2402x over previous
"""Performer linear attention (nn_PerformerLinearAttention) — Trainium2 Bass kernel.

Sharding: 8 cores = (batch 2) x (sequence 4); core c handles batch c//4,
positions [(c%4)*1024, (c%4+1)*1024). Chunked causal linear attention with
chunk size 128; cross-core KV/z prefix state via an AllGather (groups
[[0..3],[4..7]]) of per-core KV totals, combined on-device with per-core
prefix-weight inputs (rank-agnostic program).

Compute dtype: bf16 matmul streams with fp32 PSUM accumulation; the
denominator path (feature sums, cumulative z scan, reciprocal) and the
collective payload stay fp32.

Layout notes:
- GQA pair g (q heads 2g, 2g+1 share kv head g). Storage is parity-packed:
  j = g//2, p = g%2; partition rows [64p, 64p+64) of the "j" tile hold pair g.
  * qq[j]  [128, 2048]: q features, head hh of pair g at cols [hh*1024, ...)
  * kk[j]  [128, 1024]: k features (f-major)
  * snaps[j][c] [128, 64], KVg[j] [128, 64]
- Attention matmuls batch the two q-heads of a pair along the free dim
  (N=256) via 3-level APs.

Phase order (program order ~ Tile priority):
  0 consts/hT/weights loads     1 v projection (pos-major)
  2 k path: proj+rope (transient), k features -> kk, ksums, kp_pos,
    KV chains + snapshots, cc_in DMAs
  3 ksum scan, z, AllGather (fires early; later phases overlap it)
  4 per pair g: q proj+rope+features+qsums, then attention
    (masked AT, numerator partial into attnT)
  5 prefix assembly + denominators
  6 per half: pass 2 (global KV term + normalize) then output projection
"""
import numpy as np
import ml_dtypes

import concourse.bacc as bacc
import concourse.mybir as mybir
import concourse.tile as tile
from concourse import bass_utils

FP32 = mybir.dt.float32
BF16 = mybir.dt.bfloat16
CDT = BF16                      # compute dtype for matmul streams
CNP = ml_dtypes.bfloat16        # numpy equivalent
ADD = mybir.AluOpType.add
MULT = mybir.AluOpType.mult
MAX = mybir.AluOpType.max
BYPASS = mybir.AluOpType.bypass

NH, NKV, HD, NF = 16, 8, 64, 64
EPS_K, EPS_D = 1e-4, 1e-6
B, S, HM = 2, 4096, 1024
NCORES, GROUP = 8, 4
NPOS = S // GROUP            # 1024 positions per core
CH = 128
NCH = NPOS // CH             # 8 chunks
NPAIR = NH // 2              # 8 GQA pairs == kv heads


def build_nc():
    nc = bacc.Bacc("TRN2", target_bir_lowering=False, debug=False, num_devices=NCORES)

    # per-core inputs
    hT_d = nc.dram_tensor("hT", [HM, NPOS], CDT, kind="ExternalInput")
    cosT_d = nc.dram_tensor("cosT", [128, NPOS], CDT, kind="ExternalInput")
    sinTs_d = nc.dram_tensor("sinTs", [128, NPOS], CDT, kind="ExternalInput")
    wpref_d = nc.dram_tensor("wpref", [128, GROUP], FP32, kind="ExternalInput")
    # shared inputs
    wqkT_d = nc.dram_tensor("wqkT", [HM, 1536], CDT, kind="ExternalInput")
    wvT_d = nc.dram_tensor("wvT", [HM, 512], CDT, kind="ExternalInput")
    woT_d = nc.dram_tensor("woT", [HM, HM], CDT, kind="ExternalInput")
    projS2_d = nc.dram_tensor("projS2", [128, NF], CDT, kind="ExternalInput")
    permP_d = nc.dram_tensor("permP", [128, 128], CDT, kind="ExternalInput")
    tri2_d = nc.dram_tensor("tri2", [128, 256], CDT, kind="ExternalInput")
    ones128_d = nc.dram_tensor("ones128", [128, 1], CDT, kind="ExternalInput")
    # output
    out_d = nc.dram_tensor("outp", [NPOS, HM], FP32, kind="ExternalOutput")

    with tile.TileContext(nc) as tc:
        with (
            tc.tile_pool(name="consts", bufs=1) as cp,
            tc.tile_pool(name="main", bufs=1) as mp,
            tc.tile_pool(name="trans", bufs=3) as tp,
            tc.tile_pool(name="dram", bufs=1, space="DRAM") as dp,
            tc.tile_pool(name="psA", bufs=2, space="PSUM") as psA,
            tc.tile_pool(name="psB", bufs=5, space="PSUM") as psB,
            tc.tile_pool(name="psKV", bufs=1, space="PSUM") as psKV,
        ):
            # ---- constants ----
            def const(name, shape, dt, src):
                t = cp.tile(shape, dt, tag=name, name=name)
                nc.sync.dma_start(t[:], src[:])
                return t

            cosT = const("cosT", [128, NPOS], CDT, cosT_d)
            sinTs = const("sinTs", [128, NPOS], CDT, sinTs_d)
            projS2 = const("projS2", [128, NF], CDT, projS2_d)
            permP = const("permP", [128, 128], CDT, permP_d)
            tri2 = const("tri2", [128, 256], CDT, tri2_d)
            ones128 = const("ones128", [128, 1], CDT, ones128_d)
            wpref = const("wpref", [128, GROUP], FP32, wpref_d)

            # ---- persistent arrays ----
            hTs = []
            for m in range(8):
                t = mp.tile([128, NPOS], CDT, tag=f"hT{m}", name=f"hT{m}")
                nc.sync.dma_start(t[:], hT_d[m * 128:(m + 1) * 128, :])
                hTs.append(t)
            wqk = []
            for m in range(8):
                t = mp.tile([128, 1536], CDT, tag=f"wqk{m}", name=f"wqk{m}")
                nc.sync.dma_start(t[:], wqkT_d[m * 128:(m + 1) * 128, :])
                wqk.append(t)
            qq = [mp.tile([128, 2 * NPOS], CDT, tag=f"qq{j}", name=f"qq{j}")
                  for j in range(4)]
            kk = [mp.tile([128, NPOS], CDT, tag=f"kk{j}", name=f"kk{j}")
                  for j in range(4)]
            v_sb = [mp.tile([128, 512], CDT, tag=f"v{c}", name=f"v{c}")
                    for c in range(NCH)]
            snaps = [[mp.tile([128, 64], CDT, tag=f"snap{j}_{c}",
                              name=f"snap{j}_{c}") for c in range(NCH)]
                     for j in range(4)]
            attnT = [mp.tile([128, NPOS], CDT, tag=f"attnT{p}", name=f"attnT{p}")
                     for p in range(NPAIR)]
            wo = []
            for t_ in range(8):
                w = mp.tile([128, NPOS], CDT, tag=f"wo{t_}", name=f"wo{t_}")
                nc.sync.dma_start(w[:], woT_d[t_ * 128:(t_ + 1) * 128, :])
                wo.append(w)
            kz8 = mp.tile([8, NPOS], FP32, tag="kz8")    # ksum -> ztot (in-place)
            qre = mp.tile([8, NPOS], FP32, tag="qre")    # qsum_e -> r_e (in-place)
            qro = mp.tile([8, NPOS], FP32, tag="qro")    # qsum_o -> r_o (in-place)
            KVg = [mp.tile([128, 64], CDT, tag=f"kvg{j}", name=f"kvg{j}")
                   for j in range(4)]
            zgt = mp.tile([8, GROUP], FP32, tag="zgt")
            zpref = mp.tile([8, 1], FP32, tag="zpref")

            cc_in = dp.tile([520, 64], FP32, tag="cc_in")
            cc_out = dp.tile([GROUP * 520, 64], FP32, tag="cc_out")

            # ---------- helpers ----------
            def proj_pair(rb, dest):
                """QKV projection + RoPE for row-block rb (one head pair).
                Writes the rope'd pair into `dest` [128, 1024] (CDT)."""
                for half in range(2):
                    ps = psA.tile([128, 512], FP32, tag="acc")
                    for m in range(8):
                        nc.tensor.matmul(ps[:], wqk[m][:, rb * 128:(rb + 1) * 128],
                                         hTs[m][:, half * 512:(half + 1) * 512],
                                         start=(m == 0), stop=(m == 7))
                    nc.scalar.copy(dest[:, half * 512:(half + 1) * 512], ps[:])
                for half in range(2):
                    hs = slice(half * 512, (half + 1) * 512)
                    rps = psB.tile([128, 512], FP32, tag="ps")
                    nc.tensor.matmul(rps[:], permP[:], dest[:, hs],
                                     start=True, stop=True)
                    tmp = tp.tile([128, 512], CDT, tag="ropetmp")
                    nc.vector.tensor_tensor(out=tmp[:], in0=dest[:, hs],
                                            in1=cosT[:, hs], op=MULT)
                    nc.vector.tensor_tensor(out=rps[:], in0=rps[:],
                                            in1=sinTs[:, hs], op=MULT)
                    nc.vector.tensor_tensor(out=dest[:, hs], in0=tmp[:],
                                            in1=rps[:], op=ADD)

            def row_sum(dst_row_ap, rhs_aps, bases):
                """dst row [1, 1024] (one DMA) = column sums of two [64, 512]."""
                stage = tp.tile([1, NPOS], FP32, tag="rowstage")
                for half, (rhs_ap, base) in enumerate(zip(rhs_aps, bases)):
                    sps = psB.tile([1, 512], FP32, tag="ps")
                    nc.tensor.matmul(sps[:], ones128[base:base + 64, :], rhs_ap,
                                     start=True, stop=True)
                    nc.scalar.copy(stage[:, half * 512:(half + 1) * 512], sps[:])
                nc.sync.dma_start(dst_row_ap, stage[:])

            # ---------- phase 1: v projection ----------
            with nc.named_scope("vproj"):
                wv = []
                for m in range(8):
                    t = mp.tile([128, 512], CDT, tag=f"wv{m}", name=f"wv{m}")
                    nc.sync.dma_start(t[:], wvT_d[m * 128:(m + 1) * 128, :])
                    wv.append(t)
                for c in range(NCH):
                    cs = slice(c * 128, (c + 1) * 128)
                    ps = psA.tile([128, 512], FP32, tag="acc")
                    for m in range(8):
                        nc.tensor.matmul(ps[:], hTs[m][:, cs], wv[m][:],
                                         start=(m == 0), stop=(m == 7))
                    nc.scalar.copy(v_sb[c][:], ps[:])

            # ---------- phase 2: k path ----------
            with nc.named_scope("kpath"):
                for i in range(4):          # rb = 8 + i ; kv heads 2i, 2i+1
                    kr = tp.tile([128, NPOS], CDT, tag="krot", bufs=2)
                    proj_pair(8 + i, kr)
                    for hh in range(2):
                        g = 2 * i + hh
                        j, par = g // 2, g % 2
                        base = hh * 64
                        hsl = slice(base, base + 64)
                        psl = slice(par * 64, par * 64 + 64)
                        # k features (f-major) into kk[j] parity rows
                        for half in range(2):
                            hs = slice(half * 512, (half + 1) * 512)
                            fps = psB.tile([64, 512], FP32, tag="ps")
                            nc.tensor.matmul(fps[:], projS2[hsl, :], kr[hsl, hs],
                                             start=True, stop=True)
                            nc.vector.tensor_scalar_max(kk[j][psl, hs], fps[:], 0.0)
                        row_sum(kz8[g:g + 1, :],
                                (kk[j][psl, 0:512], kk[j][psl, 512:1024]),
                                (par * 64, par * 64))
                        # kp_pos + KV chain
                        kv_ps = psKV.tile([64, 64], FP32, tag="kv")
                        for c in range(NCH):
                            cs = slice(c * 128, (c + 1) * 128)
                            pps = psB.tile([128, 64], FP32, tag="ps")
                            nc.tensor.matmul(pps[:], kr[hsl, cs], projS2[hsl, :],
                                             start=True, stop=True)
                            kp_sb = tp.tile([128, 64], CDT, tag="kpos")
                            nc.vector.tensor_scalar_max(kp_sb[:], pps[:], 0.0)
                            nc.tensor.matmul(kv_ps[:], kp_sb[:],
                                             v_sb[c][:, g * 64:(g + 1) * 64],
                                             start=(c == 0), stop=(c == NCH - 1))
                            nc.scalar.copy(snaps[j][c][psl, :], kv_ps[:])
                        kvtot = tp.tile([64, 64], FP32, tag="kvtot")
                        nc.scalar.copy(kvtot[:], kv_ps[:])
                        nc.sync.dma_start(cc_in[g * 64:(g + 1) * 64, :], kvtot[:])

            # ---------- phase 3: scan + collective ----------
            with nc.named_scope("scan_cc"):
                nc.vector.tensor_tensor_scan(
                    kz8[:, 0:512], kz8[:, 0:512], kz8[:, 0:512],
                    0.0, op0=ADD, op1=BYPASS)
                nc.vector.tensor_tensor_scan(
                    kz8[:, 512:1024], kz8[:, 512:1024], kz8[:, 512:1024],
                    kz8[:, 511:512], op0=ADD, op1=BYPASS)
                ztile = tp.tile([8, 1], FP32, tag="ztile")
                nc.vector.tensor_copy(ztile[:], kz8[:, NPOS - 1:NPOS])
                nc.sync.dma_start(cc_in[512:520, 0:1], ztile[:])
                nc.gpsimd.collective_compute(
                    "AllGather", BYPASS,
                    ins=[cc_in[:].opt()], outs=[cc_out[:].opt()],
                    replica_groups=[[0, 1, 2, 3], [4, 5, 6, 7]])

            # ---------- phase 4: q path + attention, per pair ----------
            with nc.named_scope("qattn"):
                for g in range(NPAIR):
                    j, par = g // 2, g % 2
                    psl = slice(par * 64, par * 64 + 64)
                    qr = tp.tile([128, NPOS], CDT, tag="qrot", bufs=3)
                    proj_pair(g, qr)
                    for hh in range(2):
                        hsl = slice(hh * 64, hh * 64 + 64)
                        for half in range(2):
                            hs = slice(half * 512, (half + 1) * 512)
                            fps = psB.tile([64, 512], FP32, tag="ps")
                            nc.tensor.matmul(fps[:], projS2[hsl, :], qr[hsl, hs],
                                             start=True, stop=True)
                            nc.vector.tensor_scalar(
                                qq[j][psl, hh * NPOS + half * 512:
                                      hh * NPOS + (half + 1) * 512],
                                fps[:], 0.0, EPS_K, op0=MAX, op1=ADD)
                    for hh in range(2):
                        qdst = qre if hh == 0 else qro
                        row_sum(qdst[g:g + 1, :],
                                (qq[j][psl, hh * NPOS:hh * NPOS + 512],
                                 qq[j][psl, hh * NPOS + 512:hh * NPOS + 1024]),
                                (par * 64, par * 64))
                    # attention for this pair: heads batched along free (N=256)
                    qp16 = qq[j][psl, :].rearrange("p (a b) -> p a b", b=128)
                    for c in range(NCH):
                        cs = slice(c * 128, (c + 1) * 128)
                        rhs2 = qp16[:, c:c + 9:8, :]          # [64, 2, 128]
                        at = psB.tile([128, 256], FP32, tag="ps")
                        nc.tensor.matmul(at[:], kk[j][psl, cs], rhs2,
                                         start=True, stop=True)
                        ATm = tp.tile([128, 256], CDT, tag="atm")
                        nc.vector.tensor_tensor(out=ATm[:], in0=at[:],
                                                in1=tri2[:], op=MULT)
                        nps = psB.tile([64, 256], FP32, tag="ps")
                        nc.tensor.matmul(nps[:], v_sb[c][:, g * 64:(g + 1) * 64],
                                         ATm[:], start=True, stop=(c == 0))
                        if c > 0:
                            nc.tensor.matmul(nps[:], snaps[j][c - 1][psl, :], rhs2,
                                             start=False, stop=True)
                        for hh in range(2):
                            nc.scalar.copy(
                                attnT[g][hh * 64:(hh + 1) * 64, cs],
                                nps[:, hh * 128:(hh + 1) * 128])

            # ---------- phase 5: prefix assembly + denominators ----------
            with nc.named_scope("prefden"):
                for jj in range(4):
                    kvacc = tp.tile([128, 64], FP32, tag="kvacc", bufs=2)
                    for rho in range(GROUP):
                        gt = tp.tile([128, 64], FP32, tag="gath")
                        base_r = rho * 520 + jj * 128
                        nc.gpsimd.dma_start(gt[:], cc_out[base_r:base_r + 128, :])
                        if rho == 0:
                            nc.vector.tensor_scalar_mul(kvacc[:], gt[:],
                                                        wpref[:, 0:1])
                        elif rho < GROUP - 1:
                            nc.vector.scalar_tensor_tensor(
                                out=kvacc[:], in0=gt[:],
                                scalar=wpref[:, rho:rho + 1],
                                in1=kvacc[:], op0=MULT, op1=ADD)
                        else:
                            nc.vector.scalar_tensor_tensor(
                                out=KVg[jj][:], in0=gt[:],
                                scalar=wpref[:, rho:rho + 1],
                                in1=kvacc[:], op0=MULT, op1=ADD)
                for rho in range(GROUP):
                    nc.gpsimd.dma_start(zgt[:, rho:rho + 1],
                                        cc_out[rho * 520 + 512:rho * 520 + 520, 0:1])
                nc.vector.tensor_scalar_mul(zpref[:], zgt[:, 0:1], wpref[0:8, 0:1])
                for rho in range(1, GROUP):
                    nc.vector.scalar_tensor_tensor(
                        out=zpref[:], in0=zgt[:, rho:rho + 1],
                        scalar=wpref[0:8, rho:rho + 1], in1=zpref[:],
                        op0=MULT, op1=ADD)
                nc.vector.tensor_scalar_add(kz8[:], kz8[:], zpref[:, 0:1])
                for qt in (qre, qro):
                    nc.vector.tensor_tensor(out=qt[:], in0=qt[:], in1=kz8[:],
                                            op=MULT)
                    nc.vector.tensor_scalar_add(qt[:], qt[:], EPS_D)
                    nc.vector.reciprocal(qt[:], qt[:])

            # ---------- phase 6: per half, pass 2 then output projection ----------
            for half in range(2):
                hs = slice(half * 512, (half + 1) * 512)
                with nc.named_scope("pass2"):
                    for g in range(NPAIR):
                        j, par = g // 2, g % 2
                        psl = slice(par * 64, par * 64 + 64)
                        for hh in range(2):
                            hsl = slice(hh * 64, hh * 64 + 64)
                            rtile = qre if hh == 0 else qro
                            rst = tp.tile([1, 512], FP32, tag="rst")
                            nc.sync.dma_start(rst[:], rtile[g:g + 1, hs])
                            rbc = tp.tile([128, 512], FP32, tag="rbc", bufs=2)
                            nc.gpsimd.partition_broadcast(rbc[:], rst[:])
                            kvp = psB.tile([64, 512], FP32, tag="ps")
                            nc.tensor.matmul(
                                kvp[:], KVg[j][psl, :],
                                qq[j][psl, hh * NPOS + half * 512:
                                      hh * NPOS + (half + 1) * 512],
                                start=True, stop=True)
                            dst = attnT[g][hsl, hs]
                            nc.vector.tensor_tensor(out=dst, in0=dst, in1=kvp[:],
                                                    op=ADD)
                            nc.vector.tensor_tensor(out=dst, in0=dst,
                                                    in1=rbc[hsl, :], op=MULT)
                with nc.named_scope("oproj"):
                    for c in range(4 * half, 4 * half + 4):
                        cs = slice(c * 128, (c + 1) * 128)
                        for mh in range(2):
                            ops_ = psA.tile([128, 512], FP32, tag="acc")
                            for t_ in range(8):
                                nc.tensor.matmul(ops_[:], attnT[t_][:, cs],
                                                 wo[t_][:, mh * 512:(mh + 1) * 512],
                                                 start=(t_ == 0), stop=(t_ == 7))
                            ost = tp.tile([128, 512], FP32, tag="ost")
                            nc.scalar.copy(ost[:], ops_[:])
                            nc.sync.dma_start(out_d[cs, mh * 512:(mh + 1) * 512],
                                              ost[:])

    nc.finalize()
    return nc


def _host_prep(cos, sin, W_qkv, W_o, proj):
    ratio = (NF ** -0.5) * (HD ** -0.5 + EPS_K)
    projS = (proj * ratio).T.astype(CNP)                               # [d, f]
    projS2 = np.ascontiguousarray(np.concatenate([projS, projS], axis=0))
    wqkT = np.ascontiguousarray(W_qkv[:1536].T.astype(CNP))            # [1024, 1536]
    wvT = np.ascontiguousarray(W_qkv[1536:].T.astype(CNP))             # [1024, 512]
    woT = np.ascontiguousarray(W_o.T.astype(CNP))                      # [1024, 1024]
    sgn = np.concatenate([-np.ones(32, np.float32), np.ones(32, np.float32)])
    cosT1 = cos.T.astype(np.float32)
    sinT1 = (sin.T * sgn[:, None]).astype(np.float32)
    cosT = np.concatenate([cosT1, cosT1], axis=0).astype(CNP)          # [128, S]
    sinTs = np.concatenate([sinT1, sinT1], axis=0).astype(CNP)
    P = np.zeros((HD, HD), np.float32)
    for d in range(HD):
        P[(d + 32) % HD, d] = 1.0
    permP = np.zeros((128, 128), np.float32)
    permP[:64, :64] = P
    permP[64:, 64:] = P
    tri = np.triu(np.ones((CH, CH), np.float32))
    tri2 = np.concatenate([tri, tri], axis=1).astype(CNP)              # [128, 256]
    ones128 = np.ones((128, 1), CNP)
    return dict(projS2=projS2, wqkT=wqkT, wvT=wvT, woT=woT, cosT=cosT,
                sinTs=sinTs, permP=permP.astype(CNP), tri2=tri2, ones128=ones128)


_NC_CACHE = []


def kernel(**inputs):
    hidden = np.asarray(inputs["hidden_states"], dtype=np.float32)
    cos = np.asarray(inputs["cos"], dtype=np.float32)
    sin = np.asarray(inputs["sin"], dtype=np.float32)
    W_qkv = np.asarray(inputs["W_qkv"], dtype=np.float32)
    W_o = np.asarray(inputs["W_o"], dtype=np.float32)
    proj = np.asarray(inputs["proj"], dtype=np.float32)

    prep = _host_prep(cos, sin, W_qkv, W_o, proj)
    shared = {k: prep[k] for k in ("wqkT", "wvT", "woT", "projS2", "permP",
                                   "tri2", "ones128")}

    if not _NC_CACHE:
        _NC_CACHE.append(build_nc())
    nc = _NC_CACHE[0]

    in_maps = []
    for c in range(NCORES):
        b, rho = c // GROUP, c % GROUP
        sl = slice(rho * NPOS, (rho + 1) * NPOS)
        hT = np.ascontiguousarray(hidden[b, sl].T.astype(CNP))
        wpref = np.zeros((128, GROUP), np.float32)
        wpref[:, :rho] = 1.0
        in_maps.append({"hT": hT,
                        "cosT": np.ascontiguousarray(prep["cosT"][:, sl]),
                        "sinTs": np.ascontiguousarray(prep["sinTs"][:, sl]),
                        "wpref": wpref, **shared})

    res = bass_utils.run_bass_kernel_spmd(nc, in_maps, core_ids=list(range(NCORES)))

    out = np.empty((B, S, HM), np.float32)
    for c in range(NCORES):
        b, rho = c // GROUP, c % GROUP
        out[b, rho * NPOS:(rho + 1) * NPOS, :] = res.results[c]["outp"]
    return out
